# revision 15
# baseline (speedup 1.0000x reference)
"""CFConv (SchNet continuous-filter convolution) on 8 TRN2 NeuronCores.

Strategy: shard edges by destination-node range (8 contiguous ranges of 6250
nodes) so each core owns the scatter-add for its own node range -- no
all-reduce.  Within a core, edges are host-sorted by destination and grouped
into 128-node destination blocks; the segment-sum is a sequence of one-hot
matmuls accumulating in PSUM (transposed form: lhsT = edge-major messages,
rhs = cutoff-scaled one-hot, giving feature-major aggregates directly).
h1 = h @ lin1.T is computed on device (node-rotated per core so each core's
own rows sit at fixed addresses), stored node-major bf16 in DRAM, and
fetched edge-major 512 edges at a time with non-transpose dma_gather calls
rotated across the 4 SWDGE queues (4 Q7 core pairs generate descriptors in
parallel).  The filter MLP runs layer 1 feature-major / layer 2 edge-major:
the layer-2 bias is folded into layer 1 via c1 = w2^-1 b2' (a per-partition
bias in feature-major space), so the edge-major activations need only
immediate scale/bias -- no transposes anywhere in the edge pipeline.
Softplus is computed as Exp then Ln from one activation table.  Activations
process 1024 edges per instruction to amortize the ~352-cycle ACT overhead.
Each destination block's epilogue (h2 = h1 + agg, lin2) is emitted one block
late so no in-order engine stream ever stalls on it.
"""

import sys

sys.path.insert(0, "/opt/trn_rl_repo")

import numpy as np
import ml_dtypes

import concourse.bass as bass
import concourse.mybir as mybir
import concourse.tile as tile
from concourse import bacc
from concourse import bass_utils
from concourse import hw_specs
import concourse.bacc as bacc_mod

BF16 = ml_dtypes.bfloat16
F32 = np.float32
LOG2 = float(np.log(2.0))
CUTOFF = 10.0
PI = float(np.pi)

N_NODES = 50000
N_EDGES = 800000
CH = 128
NG = 50
NCORES = 8
P = 128

dt = mybir.dt

# Route Exp/Ln to the single table that holds both, so the scalar engine
# never reloads activation tables mid-kernel.  Table ids are positional, so
# preserve dict order and only edit membership.
_orig_tables = hw_specs.get_activation_tables


def _patched_tables(arch):
    t = _orig_tables(arch)
    for name, funcs in t.items():
        if name != "natural_log_exp_and_others":
            funcs.discard(mybir.ActivationFunctionType.Exp)
            funcs.discard(mybir.ActivationFunctionType.Ln)
    return t


bacc_mod.get_activation_tables = _patched_tables


def _ceil_div(a, b):
    return -(-a // b)


def build_program(n_chp, k_blk, n_ch, n_rows_pad, nblk, gather_base=None,
                  num_devices=NCORES):
    nc = bacc.Bacc(
        "TRN2",
        target_bir_lowering=False,
        debug=False,
        enable_asserts=False,
        num_devices=num_devices,
        num_swdge_queues=4,
    )

    ne_pad = n_chp * P
    n_mac = n_chp // 16  # macro-groups of 2048 edges
    base = n_rows_pad // 2 if gather_base is None else gather_base
    nown = nblk * P  # own-node columns kept resident feature-major

    # ---- DRAM I/O ----
    h_t = nc.dram_tensor("h_t", [P, n_rows_pad], dt.bfloat16, kind="ExternalInput")
    ea_t = nc.dram_tensor("ea_t", [NG, ne_pad], dt.bfloat16, kind="ExternalInput")
    s_t = nc.dram_tensor("s_t", [P, ne_pad], dt.bfloat16, kind="ExternalInput")
    src_t = nc.dram_tensor("src_t", [P, (n_chp // 4) * 32], dt.int16,
                           kind="ExternalInput")
    w1t = nc.dram_tensor("w1t", [NG, CH], dt.bfloat16, kind="ExternalInput")
    w2t = nc.dram_tensor("w2t", [CH, CH], dt.bfloat16, kind="ExternalInput")
    lin1wt = nc.dram_tensor("lin1wt", [CH, CH], dt.bfloat16, kind="ExternalInput")
    lin2wt = nc.dram_tensor("lin2wt", [CH, CH], dt.bfloat16, kind="ExternalInput")
    b1 = nc.dram_tensor("b1", [P, 1], dt.float32, kind="ExternalInput")
    ln1b = nc.dram_tensor("ln1b", [P, 1], dt.float32, kind="ExternalInput")
    l2b = nc.dram_tensor("l2b", [P, 1], dt.float32, kind="ExternalInput")

    out_t = nc.dram_tensor("out_t", [P, nblk * P], dt.float32, kind="ExternalOutput")

    # h1 node-major staging table (bf16) for the per-edge gather
    h1d = nc.dram_tensor("h1d", [n_rows_pad, CH], dt.bfloat16, kind="Internal")

    with tile.TileContext(nc) as tc:
        with tc.tile_pool(name="cpool", bufs=1) as cpool:
            # ---- constants ----
            w1t_sb = cpool.tile([NG, CH], dt.bfloat16, tag="w1t")
            nc.sync.dma_start(out=w1t_sb[:], in_=w1t.ap())
            w2t_sb = cpool.tile([CH, CH], dt.bfloat16, tag="w2t")
            nc.sync.dma_start(out=w2t_sb[:], in_=w2t.ap())
            lin1wt_sb = cpool.tile([CH, CH], dt.bfloat16, tag="lin1wt")
            nc.sync.dma_start(out=lin1wt_sb[:], in_=lin1wt.ap())
            lin2wt_sb = cpool.tile([CH, CH], dt.bfloat16, tag="lin2wt")
            nc.sync.dma_start(out=lin2wt_sb[:], in_=lin2wt.ap())
            b1_sb = cpool.tile([P, 1], dt.float32, tag="b1")
            nc.sync.dma_start(out=b1_sb[:], in_=b1.ap())
            ln1b_sb = cpool.tile([P, 1], dt.float32, tag="ln1b")
            nc.sync.dma_start(out=ln1b_sb[:], in_=ln1b.ap())
            l2b_sb = cpool.tile([P, 1], dt.float32, tag="l2b")
            nc.sync.dma_start(out=l2b_sb[:], in_=l2b.ap())
            half_sb = cpool.tile([P, 1], dt.float32, tag="half")
            nc.gpsimd.memset(half_sb[:], 0.5)
            src_sb = cpool.tile([P, (n_chp // 4) * 32], dt.int16, tag="src")
            nc.sync.dma_start(out=src_sb[:], in_=src_t.ap())
            h1T_own = cpool.tile([P, nown], dt.bfloat16, tag="h1T_own")

            # ---- Phase A: h1 = h @ lin1.T, node-major bf16 -> h1d ----
            # 2048-row slabs: 1 input DMA, 4 psum groups of 4 blocks
            # (4 matmuls + 1 DVE copy each), 1 output DMA per slab.  Own-node
            # slabs additionally produce feature-major h1 into h1T_own.
            with (
                tc.tile_pool(name="pa", bufs=3) as pa,
                tc.tile_pool(name="ppa", bufs=2, space="PSUM") as ppa,
            ):
                for off in range(0, n_rows_pad, 2048):
                    w = min(2048, n_rows_pad - off)
                    h_sb = pa.tile([P, w], dt.bfloat16, tag="h_in")
                    nc.sync.dma_start(out=h_sb[:], in_=h_t.ap()[:, off : off + w])
                    h1_sb = pa.tile([P, w], dt.bfloat16, tag="h1_sb")
                    for g in range(0, w, 512):
                        gw = min(512, w - g)
                        nt = gw // P
                        h1_ps = ppa.tile([P, 4, P], dt.float32, tag="h1_ps")
                        for t in range(nt):
                            nc.tensor.matmul(
                                out=h1_ps[:, t, :],
                                lhsT=h_sb[:, g + t * P : g + (t + 1) * P],
                                rhs=lin1wt_sb[:],
                                start=True, stop=True,
                            )
                        # alternate PSUM drains between the (otherwise idle)
                        # scalar engine and the DVE so neither paces phase A
                        if ((off + g) // 512) % 2 == 0:
                            nc.scalar.activation(
                                out=h1_sb[:, g : g + gw],
                                in_=h1_ps[:, :nt, :].rearrange("p t c -> p (t c)"),
                                func=mybir.ActivationFunctionType.Copy,
                            )
                        else:
                            nc.vector.tensor_copy(
                                out=h1_sb[:, g : g + gw],
                                in_=h1_ps[:, :nt, :].rearrange("p t c -> p (t c)"),
                            )
                        # feature-major copy of the own-node range
                        if off + g < nown:
                            ow = min(gw, nown - off - g)
                            h1T_ps = ppa.tile([P, 512], dt.float32, tag="h1T_ps")
                            nc.tensor.matmul(
                                out=h1T_ps[:, :ow],
                                lhsT=lin1wt_sb[:],
                                rhs=h_sb[:, g : g + ow],
                                start=True, stop=True,
                            )
                            nc.scalar.activation(
                                out=h1T_own[:, off + g : off + g + ow],
                                in_=h1T_ps[:, :ow],
                                func=mybir.ActivationFunctionType.Copy,
                            )
                    nc.sync.dma_start(
                        out=h1d.ap()[off : off + w, :].rearrange(
                            "(t p) c -> p t c", p=P
                        ),
                        in_=h1_sb[:].rearrange("p (t c) -> p t c", c=CH),
                    )

            # Fence: strided self-copy touching one column of every 128-row
            # block of h1d.  Its AP spans the whole table, so it RAW-depends
            # on every phase-A write, and every gather (whose AP overlaps
            # it) RAW-depends on it -- ordering gathers after the full h1
            # table is written without thousands of explicit dep edges.
            h1d_sparse = h1d.ap().rearrange("(a b) c -> a b c", b=P)[:, 0:1, 0:1]
            with nc.allow_non_contiguous_dma(reason="sparse h1d ordering fence"):
                nc.sync.dma_start(out=h1d_sparse, in_=h1d_sparse)

            # ---- Phase B: stage-skewed pipeline, 2048 edges per macro ----
            # Round m emits: DMA(m+1) | xMM+drain+gathers(m) | L1 ACTs(m-1)
            # | wMM+drain(m-2) | L2 ACTs+mult+agg(m-3).  Every scalar
            # instruction's inputs were produced a full round earlier, so the
            # scalar engine (the pacing resource) never stalls; PE/DVE/Q7
            # absorb their waits inside the round's slack.
            with (
                tc.tile_pool(name="pea", bufs=3) as pea,
                tc.tile_pool(name="pst", bufs=6) as pst,
                tc.tile_pool(name="pxf", bufs=2) as pxf,
                tc.tile_pool(name="pwf", bufs=2) as pwf,
                tc.tile_pool(name="pe12", bufs=1) as pe12,
                tc.tile_pool(name="pg", bufs=5) as pg,
                tc.tile_pool(name="pmsg", bufs=2) as pmsg,
                tc.tile_pool(name="pep", bufs=2) as pep,
                tc.tile_pool(name="psx", bufs=1, space="PSUM") as psx,
                tc.tile_pool(name="psw", bufs=1, space="PSUM") as psw,
                tc.tile_pool(name="psagg", bufs=2, space="PSUM") as psagg,
                tc.tile_pool(name="pso", bufs=1, space="PSUM") as pso,
            ):
                agg_tiles = {}
                done_blocks = []
                ea_sbs = {}
                s_sbs = {}
                xf_sbs = {}
                x_sbs = {}
                wf_sbs = {}
                w2f_sbs = {}
                h1g_sbs = {}

                def emit_epilogue(b):
                    aggT = agg_tiles.pop(b)
                    h2T_sb = pep.tile([P, CH], dt.bfloat16, tag="h2T")
                    nc.vector.tensor_tensor(
                        out=h2T_sb[:], in0=aggT[:],
                        in1=h1T_own[:, b * P : (b + 1) * P],
                        op=mybir.AluOpType.add,
                    )
                    o_ps = pso.tile([P, P], dt.float32, tag="o_ps")
                    nc.tensor.matmul(
                        out=o_ps[:], lhsT=lin2wt_sb[:], rhs=h2T_sb[:],
                        start=True, stop=True,
                    )
                    o_sb = pep.tile([P, P], dt.float32, tag="o_sb")
                    nc.vector.tensor_scalar(
                        out=o_sb[:], in0=o_ps[:],
                        scalar1=l2b_sb[:, 0:1], scalar2=None,
                        op0=mybir.AluOpType.add,
                    )
                    nc.sync.dma_start(
                        out=out_t.ap()[:, b * P : (b + 1) * P], in_=o_sb[:]
                    )

                for m in range(n_mac + 4):
                    # -- input DMA, one macro ahead --
                    if m < n_mac:
                        es = m * 2048
                        ea_sb = pea.tile([NG, 2048], dt.bfloat16, tag="ea")
                        nc.sync.dma_start(
                            out=ea_sb[:], in_=ea_t.ap()[:, es : es + 2048]
                        )
                        ea_sbs[m] = ea_sb
                        s_sb = pst.tile([P, 2048], dt.bfloat16, tag="s_sel")
                        nc.sync.dma_start(
                            out=s_sb[:], in_=s_t.ap()[:, es : es + 2048]
                        )
                        s_sbs[m] = s_sb

                    # -- S0(m-1): layer-1 matmuls, PSUM drain, gathers --
                    q = m - 1
                    if 0 <= q < n_mac:
                        ea_sb = ea_sbs.pop(q)
                        xf_sb = pxf.tile([P, 2048], dt.float32, tag="xf")
                        xf_sbs[q] = xf_sb
                        for gi in range(2):
                            x_ps = psx.tile([P, 1024], dt.float32, tag="x_ps")
                            for h in range(2):
                                nc.tensor.matmul(
                                    out=x_ps[:, h * 512 : (h + 1) * 512],
                                    lhsT=w1t_sb[:],
                                    rhs=ea_sb[:, gi * 1024 + h * 512 :
                                              gi * 1024 + (h + 1) * 512],
                                    start=True, stop=True,
                                )
                            nc.vector.tensor_copy(
                                out=xf_sb[:, gi * 1024 : (gi + 1) * 1024],
                                in_=x_ps[:],
                            )
                        # edge-major gathers; pads alias a real row so every
                        # int16 window ends non-negative and nothing is
                        # skipped.  One gather per SWDGE queue per macro.
                        h1g_sb = pg.tile([P, 16, P], dt.bfloat16, tag="h1g")
                        h1g_sbs[q] = h1g_sb
                        for h in range(4):
                            nc.gpsimd.dma_gather(
                                out_ap=h1g_sb[:, 4 * h : 4 * h + 4, :],
                                in_ap=h1d.ap()[base:, :],
                                idxs_ap=src_sb[:, (4 * q + h) * 32 :
                                               (4 * q + h + 1) * 32],
                                num_idxs=512,
                                num_idxs_reg=512,
                                elem_size=CH,
                                transpose=False,
                                queue_num=h,
                            )

                    # -- S1(m-2): layer-1 activations (feature-major) --
                    # x' = c1 + softplus(y1+b1) = ln(exp(y1+b1+c1) + e^c1)
                    q = m - 2
                    if 0 <= q < n_mac:
                        xf_sb = xf_sbs.pop(q)
                        e1_sb = pe12.tile([P, 2048], dt.float32, tag="e1")
                        nc.scalar.activation(
                            out=e1_sb[:], in_=xf_sb[:],
                            func=mybir.ActivationFunctionType.Exp,
                            bias=b1_sb[:, 0:1],
                        )
                        x_sb = pxf.tile([P, 2048], dt.bfloat16, tag="x_sb")
                        x_sbs[q] = x_sb
                        nc.scalar.activation(
                            out=x_sb[:], in_=e1_sb[:],
                            func=mybir.ActivationFunctionType.Ln,
                            bias=ln1b_sb[:, 0:1],
                        )

                    # -- S2(m-3): layer-2 matmuls (edge-major) + drain --
                    q = m - 3
                    if 0 <= q < n_mac:
                        x_sb = x_sbs.pop(q)
                        wf_sb = pwf.tile([P, 2048], dt.float32, tag="wf")
                        wf_sbs[q] = wf_sb
                        for gi in range(2):
                            w_ps = psw.tile([P, 8, P], dt.float32, tag="w_ps")
                            for t in range(8):
                                tt = gi * 8 + t
                                nc.tensor.matmul(
                                    out=w_ps[:, t, :],
                                    lhsT=x_sb[:, tt * P : (tt + 1) * P],
                                    rhs=w2t_sb[:],
                                    start=True, stop=True,
                                )
                            nc.vector.tensor_copy(
                                out=wf_sb[:, gi * 1024 : (gi + 1) * 1024],
                                in_=w_ps[:].rearrange("p t c -> p (t c)"),
                            )

                    # -- S3+S5+S6(m-4): layer-2 ACTs, message, scatter --
                    q = m - 4
                    if 0 <= q < n_mac:
                        wf_sb = wf_sbs.pop(q)
                        e2_sb = pe12.tile([P, 2048], dt.float32, tag="e2")
                        nc.scalar.activation(
                            out=e2_sb[:], in_=wf_sb[:],
                            func=mybir.ActivationFunctionType.Exp,
                        )
                        w2f_sb = pwf.tile([P, 2048], dt.bfloat16, tag="w2f")
                        nc.scalar.activation(
                            out=w2f_sb[:], in_=e2_sb[:],
                            func=mybir.ActivationFunctionType.Ln,
                            bias=half_sb[:, 0:1],
                            scale=0.5,
                        )
                        h1g_sb = h1g_sbs.pop(q)
                        msg_sb = pmsg.tile([P, 16, P], dt.bfloat16, tag="msg")
                        nc.vector.tensor_tensor(
                            out=msg_sb[:].rearrange("p t c -> p (t c)"),
                            in0=w2f_sb[:],
                            in1=h1g_sb[:].rearrange("p t c -> p (t c)"),
                            op=mybir.AluOpType.mult,
                        )
                        s_sb = s_sbs.pop(q)
                        for t in range(16):
                            k = 16 * q + t
                            if k >= n_ch:
                                continue
                            b = k // k_blk
                            j = k % k_blk
                            if j == 0:
                                agg_tile = psagg.tile(
                                    [P, CH], dt.float32, tag="agg"
                                )
                                agg_tiles[b] = agg_tile
                            nc.tensor.matmul(
                                out=agg_tiles[b][:],
                                lhsT=msg_sb[:, t, :],
                                rhs=s_sb[:, t * P : (t + 1) * P],
                                start=(j == 0), stop=(j == k_blk - 1),
                            )

                            if j == k_blk - 1 and b < nblk:
                                done_blocks.append(b)
                                if len(done_blocks) >= 2:
                                    emit_epilogue(done_blocks.pop(0))

                while done_blocks:
                    emit_epilogue(done_blocks.pop(0))

    nc.compile()
    return nc


def prep_inputs(h, edge_index, edge_weight, edge_attr,
                lin1_w, nn_w1, nn_b1, nn_w2, nn_b2, lin2_w, lin2_b,
                n_nodes, ncores=NCORES, gather_base=None):
    """Host-side sharding/layout. Returns (params, in_maps, meta)."""
    npc = n_nodes // ncores
    nblk = _ceil_div(npc, P)
    # +1 guarantees a spare pad row: source id base-1 would encode to the
    # int16 gather sentinel -1, so those edges are pointed at an alias row.
    n_rows_pad = _ceil_div(n_nodes + 1, P) * P
    base = n_rows_pad // 2 if gather_base is None else gather_base
    r_star = n_rows_pad - 1

    dst = np.asarray(edge_index[0], dtype=np.int64)
    src = np.asarray(edge_index[1], dtype=np.int64)
    ne = dst.shape[0]

    order = np.argsort(dst, kind="stable")
    dsts = dst[order]
    srcs = src[order]
    ews = np.asarray(edge_weight, dtype=np.float32)[order]
    eas = np.asarray(edge_attr, dtype=np.float32)[order]
    cs = (0.5 * (np.cos(ews * (PI / CUTOFF)) + 1.0)).astype(np.float32)

    core_of = dsts // npc
    d_loc = dsts - core_of * npc
    blk = d_loc // P
    key = core_of * nblk + blk
    cnt = np.bincount(key, minlength=ncores * nblk)
    k_blk = max(1, int(_ceil_div(int(cnt.max()), P)))
    n_ch = nblk * k_blk
    n_chp = _ceil_div(n_ch, 16) * 16
    ne_pad = n_chp * P
    n_sup = n_chp // 4

    key_start = np.zeros(ncores * nblk + 1, dtype=np.int64)
    np.cumsum(cnt, out=key_start[1:])
    rank = np.arange(ne, dtype=np.int64) - key_start[key]
    pos_in_core = blk * (k_blk * P) + rank

    lo_hi = np.searchsorted(dsts, np.arange(ncores + 1) * npc)

    ht = np.zeros((P, n_rows_pad), dtype=BF16)
    ht[:, :n_nodes] = np.asarray(h, dtype=np.float32).T.astype(BF16)

    w1t_a = np.ascontiguousarray(np.asarray(nn_w1, np.float32).T).astype(BF16)
    w2t_a = np.ascontiguousarray(np.asarray(nn_w2, np.float32).T).astype(BF16)
    lin1wt_a = np.ascontiguousarray(np.asarray(lin1_w, np.float32).T).astype(BF16)
    lin2wt_a = np.ascontiguousarray(np.asarray(lin2_w, np.float32).T).astype(BF16)

    # Fold the layer-2 bias (and both -log2 softplus shifts) into layer 1:
    # find c1 with w2 @ c1 = b2 - log2 * (w2 @ 1), then
    #   x' = c1 + softplus(y1 + b1) = ln(exp(y1 + b1 + c1) + exp(c1))
    # so layer 2 needs no per-filter bias in edge-major layout.
    w2_f64 = np.asarray(nn_w2, np.float64)
    b2_f64 = np.asarray(nn_b2, np.float64)
    c1 = np.linalg.lstsq(w2_f64, b2_f64, rcond=None)[0] - LOG2
    resid = np.abs(w2_f64 @ c1 - (b2_f64 - LOG2 * w2_f64.sum(axis=1))).max()
    assert resid < 1e-6 and np.abs(c1).max() < 20.0, (resid, np.abs(c1).max())
    b1_a = (np.asarray(nn_b1, np.float64) + c1).astype(np.float32).reshape(P, 1)
    ln1b_a = np.exp(c1).astype(np.float32).reshape(P, 1)
    l2b_a = np.asarray(lin2_b, np.float32).reshape(P, 1)

    # First pass: per-core sorted source layouts.  Sorting each chunk by
    # (rotated) source id is gather-base independent; pads (= r_star, the
    # max) land in the last lanes.  The gather engine stops at the last
    # NON-NEGATIVE int16 index, so each 512-index call must end with
    # idx >= 0, i.e. base <= the call's last (max) source id.
    per_core = []
    required = r_star
    for c in range(ncores):
        lo, hi = int(lo_hi[c]), int(lo_hi[c + 1])
        pos = pos_in_core[lo:hi]
        srcv = (srcs[lo:hi] - c * npc) % n_nodes
        src_pad = np.full(ne_pad, r_star, dtype=np.int64)  # pads -> alias row
        src_pad[pos] = srcv
        perm = np.argsort(src_pad.reshape(-1, P), axis=1, kind="stable")
        flat_perm = (
            perm + (np.arange(n_chp, dtype=np.int64) * P)[:, None]
        ).ravel()
        src_pad = src_pad[flat_perm]
        required = min(required, int(src_pad.reshape(n_sup, 512)[:, -1].min()))
        per_core.append((lo, hi, pos, src_pad, flat_perm))

    if gather_base is None:
        base = min(n_rows_pad // 2, required)
        base_floor = max(0, n_rows_pad - 1 - 32767)
        assert base >= base_floor, (
            f"cannot pick an int16 gather base: need <= {required}, "
            f">= {base_floor}"
        )

    in_maps = []
    for c in range(ncores):
        lo, hi, pos, src_pad, flat_perm = per_core[c]
        if base > 0:
            # source id base-1 would encode to the int16 sentinel -1;
            # point those edges at the alias row instead (same features).
            src_pad = np.where(src_pad == base - 1, r_star, src_pad)
        idx16 = (src_pad - base).astype(np.int16)
        assert (idx16.reshape(n_sup, 512)[:, -1] >= 0).all()
        idx_w = idx16.reshape(n_sup, 32, 16)
        idx_w = np.transpose(idx_w, (0, 2, 1))               # [n_sup, 16, 32]
        src_a = np.ascontiguousarray(
            np.tile(idx_w, (1, 8, 1)).transpose(1, 0, 2).reshape(P, n_sup * 32)
        )

        # position of each real edge after the within-chunk permutation
        inv_perm = np.empty(ne_pad, dtype=np.int64)
        inv_perm[flat_perm] = np.arange(ne_pad, dtype=np.int64)
        pos2 = inv_perm[pos]

        # dense cutoff-scaled one-hot selection matrices, [P, n_chp*128]
        s_all = np.zeros((P, ne_pad), dtype=BF16)
        lane = pos2 % P
        chunk = pos2 // P
        dstl = d_loc[lo:hi] - blk[lo:hi] * P
        s_all[lane, chunk * P + dstl] = cs[lo:hi].astype(BF16)

        ea_pad = np.zeros((ne_pad, NG), dtype=BF16)
        ea_pad[pos2] = eas[lo:hi].astype(BF16)

        htc = np.concatenate(
            [ht[:, c * npc : n_nodes], ht[:, : c * npc], ht[:, n_nodes:]], axis=1
        )
        if base > 0:
            htc[:, r_star] = htc[:, base - 1]

        in_maps.append({
            "h_t": np.ascontiguousarray(htc),
            "ea_t": np.ascontiguousarray(ea_pad.T),
            "s_t": s_all,
            "src_t": src_a,
            "w1t": w1t_a,
            "w2t": w2t_a,
            "lin1wt": lin1wt_a,
            "lin2wt": lin2wt_a,
            "b1": b1_a,
            "ln1b": ln1b_a,
            "l2b": l2b_a,
        })

    params = dict(n_chp=n_chp, k_blk=k_blk, n_ch=n_ch,
                  n_rows_pad=n_rows_pad, nblk=nblk, gather_base=base)
    meta = dict(npc=npc, n_nodes=n_nodes, ncores=ncores)
    return params, in_maps, meta


def assemble_output(results, meta):
    npc = meta["npc"]
    n_nodes = meta["n_nodes"]
    out = np.empty((n_nodes, CH), dtype=np.float32)
    for c in range(meta["ncores"]):
        out[c * npc : (c + 1) * npc] = results[c]["out_t"][:, :npc].T
    return out


def kernel(**inputs):
    params, in_maps, meta = prep_inputs(
        inputs["h"], inputs["edge_index"], inputs["edge_weight"],
        inputs["edge_attr"], inputs["lin1_w"], inputs["nn_w1"],
        inputs["nn_b1"], inputs["nn_w2"], inputs["nn_b2"],
        inputs["lin2_w"], inputs["lin2_b"], N_NODES,
    )
    nc = build_program(**params)

    # The DGE gather stream very occasionally corrupts a 512-edge window
    # (and a crashed device can silently corrupt the next run), so execute
    # until two runs agree bit-exactly.
    last_err = None
    outputs = []
    for _attempt in range(6):
        try:
            br = bass_utils.run_bass_kernel_spmd(
                nc, in_maps, core_ids=list(range(NCORES))
            )
        except Exception as e:  # transient device errors: retry
            last_err = e
            continue
        out = assemble_output(br.results, meta)
        for prev in outputs:
            if np.array_equal(prev, out):
                return out
        outputs.append(out)
    if outputs:
        return outputs[-1]
    raise last_err
